# revision 1
# baseline (speedup 1.0000x reference)
"""Trainium2 Bass kernel for a 2-layer GCN (PyG GCNConv semantics) + linear head.

Strategy (8 NeuronCores, SPMD):
  - Nodes are sharded across cores by id: core c owns rows [c*6250, (c+1)*6250),
    padded to 6272 = 49*128 local rows.
  - Edges are bucketed by dst shard (host-side sort), grouped per 128-node dst
    tile, packed into uniform 128-edge blocks (equal per core/tile via
    zero-weight padding edges, so one SPMD program fits all cores).
  - Per layer: each core computes h = x_shard @ W on its own slice, scales by
    dinv (g = dinv * h), casts to bf16 and AllGathers the g-table to DRAM.
    Aggregation fetches g[src] rows with the Q7 dma_gather (int16 indices; the
    50176-row table is addressed as two 25088-row halves, with edges packed
    into half-pure blocks) and reduces them per dst tile with a one-hot
    "segment matrix" matmul on the tensor engine:
       S_w[e, n] = ew[e] * (dst_local[e] == n)      (one DVE tensor_scalar op)
       psum[n, f] += S_w^T @ g_gathered[e, f]        (PE matmul, PSUM accum)
    The self-loop term and the bias are folded in as synthetic blocks per tile
    (self: ew=1 on the owning half, 0 on the other; bias: scalar2 = sqrt(deg),
    bias row stashed in a phantom table row), so the epilogue is a single
    relu(dinv * psum) activation.
  - Head: out^T = Wc^T @ relu(h2)^T per 512-column chunk, + bc, one DMA out.

Host side does only integer/layout work (sort, bucket, pad, transpose-pack,
index translation); all floating-point math runs on device.
"""

import sys

import numpy as np

for _p in ("/opt/trn_rl_repo",):
    if _p not in sys.path:
        sys.path.append(_p)

import ml_dtypes

import concourse.bacc as bacc
import concourse.mybir as mybir
import concourse.tile as tile
from concourse.bass_utils import run_bass_kernel_spmd

BF16 = ml_dtypes.bfloat16

N_NODES = 50000
N_EDGES = 600000
D = 128
N_CLS = 10
N_CORES = 8
NPC = N_NODES // N_CORES  # 6250
P = 128
T_PER_CORE = (NPC + P - 1) // P  # 49
NPC_PAD = T_PER_CORE * P  # 6272
N_TAB = N_CORES * NPC_PAD  # 50176
SPLIT = 4 * NPC_PAD  # 25088: table half boundary (fits int16 indexing)
BIAS_ROW = NPC_PAD - 1  # phantom row in half A carrying the layer bias
GCH = 4  # dst tiles per gather chunk

f32 = mybir.dt.float32
bf16 = mybir.dt.bfloat16
i16 = mybir.dt.int16

f32n = np.float32


def _table_row(global_node):
    """Row in the AllGather'd g-table for a global node id. Per-core AllGather
    contribution is the SBUF-ordered [128, 49, 128] g staging tile, i.e. local
    row r = t*128 + p lands at flat c*6272 + p*49 + t."""
    c = global_node // NPC
    r = global_node % NPC
    return c * NPC_PAD + (r % P) * T_PER_CORE + r // P


def _pad32(n):
    return -(-n // 32) * 32


def _layout(TBA, TBB):
    """Static program layout for given per-half block counts.

    Per-tile column order: [A-data x TBA | selfA | bias | B-data x TBB | selfB]
    (first TBA+2 cols gather from table half A, last TBB+1 from half B).
    Gather calls are per (chunk of GCH tiles, half). Returns layout dict.
    """
    CA = TBA + 2
    CB = TBB + 1
    cols = CA + CB
    chunks = []
    pos = 0
    for t0 in range(0, T_PER_CORE, GCH):
        tcnt = min(GCH, T_PER_CORE - t0)
        a16 = pos
        pos += _pad32(tcnt * CA * 8)  # int16 cols per call, 64B-aligned
        b16 = pos
        pos += _pad32(tcnt * CB * 8)
        chunks.append((t0, tcnt, a16, b16))
    return dict(CA=CA, CB=CB, cols=cols, chunks=chunks, idxw=pos)


def _pack_edges(edge_index, edge_weight):
    """Returns (TBA, TBB, idx16, dstl_cols, ew_cols).

    idx16    : int16 [8, 128, idxw] dma_gather index tiles (x8 replicated rows)
    dstl_cols: f32   [8, 128, 49*cols] local dst id within tile
    ew_cols  : f32   [8, 128, 49*cols] S_w scalar2 source (edge weight; 1/0 for
               self cols; unused for the bias col)
    """
    src = edge_index[0].astype(np.int64)
    dst = edge_index[1].astype(np.int64)
    ew = np.asarray(edge_weight, f32n)

    tr = _table_row(src)
    half = (tr >= SPLIT).astype(np.int64)

    core_of = dst // NPC
    rloc = dst % NPC
    tile_of = rloc // P
    dstl_all = (rloc % P).astype(f32n)

    group = (core_of * T_PER_CORE + tile_of) * 2 + half
    order = np.argsort(group, kind="stable")
    ew_s = ew[order]
    tr_s = tr[order]
    group_s = group[order]
    dstl_s = dstl_all[order]

    counts = np.bincount(group_s, minlength=N_CORES * T_PER_CORE * 2)
    TBA = int(np.ceil(counts[0::2].max() / P))
    TBB = int(np.ceil(counts[1::2].max() / P))
    L = _layout(TBA, TBB)
    CA, CB, cols = L["CA"], L["CB"], L["cols"]
    ncols = T_PER_CORE * cols

    starts = np.concatenate(([0], np.cumsum(counts)[:-1]))
    within = np.arange(len(group_s)) - starts[group_s]
    blk = within // P
    lane = within % P

    core_s = group_s // (2 * T_PER_CORE)
    tile_s = (group_s // 2) % T_PER_CORE
    half_s = group_s % 2
    col = tile_s * cols + np.where(half_s == 0, blk, CA + blk)

    dstl_cols = np.zeros((N_CORES, P, ncols), f32n)
    ew_cols = np.zeros((N_CORES, P, ncols), f32n)
    rows = np.zeros((N_CORES, P, ncols), np.int32)
    dstl_cols[core_s, lane, col] = dstl_s
    ew_cols[core_s, lane, col] = ew_s
    rows[core_s, lane, col] = tr_s - half_s * SPLIT

    iota = np.arange(P)
    for c in range(N_CORES):
        own_half = 0 if c < 4 else 1
        own_tab_all = c * NPC_PAD + (np.arange(NPC_PAD) % P) * T_PER_CORE + np.arange(NPC_PAD) // P
        for t in range(T_PER_CORE):
            sa = t * cols + TBA
            bcol = t * cols + TBA + 1
            sb = t * cols + CA + TBB
            dstl_cols[c, :, sa] = iota
            dstl_cols[c, :, bcol] = iota
            dstl_cols[c, :, sb] = iota
            ew_cols[c, :, sa] = 1.0 if own_half == 0 else 0.0
            ew_cols[c, :, sb] = 1.0 if own_half == 1 else 0.0
            own_tab = own_tab_all[t * P : (t + 1) * P]
            rows[c, :, sa if own_half == 0 else sb] = own_tab - own_half * SPLIT
            rows[c, :, bcol] = BIAS_ROW

    # idx16: per (chunk, half) block [16, w]; position [p%16, cl*8 + p//16]
    # holds the gather row of (lane p, call col cl); replicated to 8 groups.
    idx16 = np.zeros((N_CORES, P, L["idxw"]), np.int16)
    pgrid = np.arange(P)[:, None]
    for c in range(N_CORES):
        for (t0, tcnt, a16, b16) in L["chunks"]:
            for hh, start, CHW in ((0, a16, CA), (1, b16, CB)):
                base = np.empty((P, tcnt * CHW), np.int32)
                for ti in range(tcnt):
                    t = t0 + ti
                    off = t * cols + (0 if hh == 0 else CA)
                    base[:, ti * CHW : (ti + 1) * CHW] = rows[c, :, off : off + CHW]
                blkarr = np.zeros((16, _pad32(tcnt * CHW * 8)), np.int16)
                cl = np.arange(tcnt * CHW)[None, :]
                blkarr[pgrid % 16, cl * 8 + pgrid // 16] = base.astype(np.int16)
                idx16[c, :, start : start + blkarr.shape[1]] = np.tile(blkarr, (8, 1))

    return TBA, TBB, idx16, dstl_cols, ew_cols


def _build_program(TBA, TBB, debug_taps=False):
    L = _layout(TBA, TBB)
    CA, CB, cols = L["CA"], L["CB"], L["cols"]
    ncols = T_PER_CORE * cols

    nc = bacc.Bacc(target_bir_lowering=False)

    xT_ext = nc.declare_dram_parameter("xT", [P, NPC_PAD], f32, isOutput=False)
    w1_ext = nc.declare_dram_parameter("W1", [D, D], f32, isOutput=False)
    w2_ext = nc.declare_dram_parameter("W2", [D, D], f32, isOutput=False)
    wc_ext = nc.declare_dram_parameter("Wc", [D, P], f32, isOutput=False)
    bc_ext = nc.declare_dram_parameter("bc", [P, 1], f32, isOutput=False)
    b1_ext = nc.declare_dram_parameter("b1row", [1, D], bf16, isOutput=False)
    b2_ext = nc.declare_dram_parameter("b2row", [1, D], bf16, isOutput=False)
    idx_ext = nc.declare_dram_parameter("idx16", [P, L["idxw"]], i16, isOutput=False)
    dstl_ext = nc.declare_dram_parameter("dstl_cols", [P, ncols], f32, isOutput=False)
    ew_ext = nc.declare_dram_parameter("ew_cols", [P, ncols], f32, isOutput=False)
    iota_ext = nc.declare_dram_parameter("iota_tile", [P, P], bf16, isOutput=False)
    ones_ext = nc.declare_dram_parameter("ones_col", [P, 1], bf16, isOutput=False)
    ident_ext = nc.declare_dram_parameter("ident", [P, P], f32, isOutput=False)
    out_ext = nc.declare_dram_parameter("outT", [P, NPC_PAD], f32, isOutput=True)
    if debug_taps:
        deg_dbg = nc.declare_dram_parameter("deg_dbg", [P, T_PER_CORE], f32, isOutput=True)
        gtab_dbg = nc.declare_dram_parameter("gtab_dbg", [N_TAB, D], bf16, isOutput=True)
        gbuf_dbg = nc.declare_dram_parameter("gbuf_dbg", [P, GCH * CA, D], bf16, isOutput=True)

    ag_in = [
        nc.dram_tensor(f"ag_in{l}", [P, T_PER_CORE, P], bf16, kind="Internal")
        for l in (1, 2)
    ]
    g_tab = [
        nc.dram_tensor(
            f"g_table{l}", [N_TAB, D], bf16, kind="Internal", addr_space="Shared"
        )
        for l in (1, 2)
    ]

    core_ids = list(range(N_CORES))

    with tile.TileContext(nc) as tc:
        with (
            tc.tile_pool(name="const", bufs=1) as cpool,
            tc.tile_pool(name="meta", bufs=1) as mpool,
            tc.tile_pool(name="big", bufs=1) as bigpool,
            tc.tile_pool(name="gatherA", bufs=3) as gpoolA,
            tc.tile_pool(name="gatherB", bufs=3) as gpoolB,
            tc.tile_pool(name="sw", bufs=4) as swpool,
            tc.tile_pool(name="work", bufs=4) as wpool,
            tc.tile_pool(name="psum_msg", bufs=2, space="PSUM") as pp_msg,
            tc.tile_pool(name="psum_h", bufs=2, space="PSUM") as pp_h,
            tc.tile_pool(name="psum_tr", bufs=1, space="PSUM") as pp_tr,
            tc.tile_pool(name="psum_cls", bufs=1, space="PSUM") as pp_cls,
            tc.tile_pool(name="psum_deg", bufs=1, space="PSUM") as pdeg,
            tc.tile_pool(name="psum_trash", bufs=1, space="PSUM") as pp_trash,
        ):
            # ---------- load constants / metadata ----------
            xT = bigpool.tile([P, NPC_PAD], f32, tag="xT")
            nc.sync.dma_start(out=xT[:], in_=xT_ext[:])
            w1 = cpool.tile([D, D], f32, tag="w1")
            nc.sync.dma_start(out=w1[:], in_=w1_ext[:])
            w2 = cpool.tile([D, D], f32, tag="w2")
            nc.sync.dma_start(out=w2[:], in_=w2_ext[:])
            wc = cpool.tile([D, P], f32, tag="wc")
            nc.sync.dma_start(out=wc[:], in_=wc_ext[:])
            bc = cpool.tile([P, 1], f32, tag="bc")
            nc.sync.dma_start(out=bc[:], in_=bc_ext[:])
            iota = cpool.tile([P, P], bf16, tag="iota")
            nc.sync.dma_start(out=iota[:], in_=iota_ext[:])
            ones = cpool.tile([P, 1], bf16, tag="ones")
            nc.sync.dma_start(out=ones[:], in_=ones_ext[:])
            ident = cpool.tile([P, P], f32, tag="ident")
            nc.sync.dma_start(out=ident[:], in_=ident_ext[:])
            idxm = mpool.tile([P, L["idxw"]], i16, tag="idxm")
            nc.sync.dma_start(out=idxm[:], in_=idx_ext[:])
            dstlm = mpool.tile([P, ncols], f32, tag="dstlm")
            nc.sync.dma_start(out=dstlm[:], in_=dstl_ext[:])
            ewm = mpool.tile([P, ncols], f32, tag="ewm")
            nc.sync.dma_start(out=ewm[:], in_=ew_ext[:])

            # PE is hardware-decoded and carries at most one semaphore wait per
            # instruction. Absorb each DMA lane's completion into PE's observed
            # clock via dummy matmuls accumulating into one never-read PSUM
            # group (group members have no WAW hazard between them).
            n_absorb = 6 + 2 * 2 * len(L["chunks"])
            trash = pp_trash.tile([1, 1], f32, tag="trash")
            absorb_state = {"i": 0}

            def pe_absorb(ap):
                i = absorb_state["i"]
                absorb_state["i"] += 1
                nc.tensor.matmul(
                    trash[:],
                    lhsT=ap,
                    rhs=ap,
                    start=(i == 0),
                    stop=(i == n_absorb - 1),
                    skip_group_check=True,
                )

            for _t in (xT, w1, w2, wc, ones, ident):
                pe_absorb(_t[:, :1] if _t.shape[1] > 1 else _t[:])

            # DVE waits are capped at two; pre-observe the metadata DMA lanes.
            for _t in (iota, dstlm, ewm):
                dabs = wpool.tile([1, 1], _t.dtype, tag="dabs")
                nc.vector.tensor_copy(out=dabs[:], in_=_t[:1, :1])

            def build_sw(c0, scalar2):
                sw = swpool.tile([P, P], bf16, tag="sw")
                nc.vector.tensor_scalar(
                    out=sw[:],
                    in0=iota[:],
                    scalar1=dstlm[:, c0 : c0 + 1],
                    scalar2=scalar2,
                    op0=mybir.AluOpType.is_equal,
                    op1=mybir.AluOpType.mult,
                )
                return sw

            # ---------- degree pass (data cols only) ----------
            deg = cpool.tile([P, T_PER_CORE], f32, tag="deg")
            for t in range(T_PER_CORE):
                pd = pdeg.tile([P, 1], f32, tag="pdeg")
                dcols = [t * cols + j for j in range(TBA)] + [
                    t * cols + CA + j for j in range(TBB)
                ]
                for jj, c0 in enumerate(dcols):
                    sw = build_sw(c0, ewm[:, c0 : c0 + 1])
                    nc.tensor.matmul(
                        pd[:],
                        lhsT=sw[:],
                        rhs=ones[:],
                        start=(jj == 0),
                        stop=(jj == len(dcols) - 1),
                    )
                nc.vector.tensor_scalar(
                    out=deg[:, t : t + 1],
                    in0=pd[:],
                    scalar1=1.0,
                    scalar2=None,
                    op0=mybir.AluOpType.add,
                )
            if debug_taps:
                nc.sync.dma_start(out=deg_dbg[:], in_=deg[:])

            recip = cpool.tile([P, T_PER_CORE], f32, tag="recip")
            nc.vector.reciprocal(out=recip[:], in_=deg[:])
            dinv = cpool.tile([P, T_PER_CORE], f32, tag="dinv")
            nc.scalar.activation(dinv[:], recip[:], mybir.ActivationFunctionType.Sqrt)
            sqd = cpool.tile([P, T_PER_CORE], f32, tag="sqd")
            nc.scalar.activation(sqd[:], deg[:], mybir.ActivationFunctionType.Sqrt)

            # ---------- layers ----------
            reluT_prev = None
            for layer in (0, 1):
                w = (w1, w2)[layer]
                b_ext = (b1_ext, b2_ext)[layer]

                gstage = bigpool.tile([P, T_PER_CORE, P], bf16, tag=f"gstage{layer}")
                for t in range(T_PER_CORE):
                    ph = pp_h.tile([P, D], f32, tag="ph")
                    lhsT = (
                        xT[:, t * P : (t + 1) * P]
                        if layer == 0
                        else reluT_prev[:, t, :]
                    )
                    nc.tensor.matmul(ph[:], lhsT=lhsT, rhs=w[:], start=True, stop=True)
                    nc.scalar.activation(
                        gstage[:, t, :],
                        ph[:],
                        mybir.ActivationFunctionType.Copy,
                        scale=dinv[:, t : t + 1],
                    )
                nc.sync.dma_start(out=ag_in[layer][:], in_=gstage[:])
                nc.sync.dma_start(
                    out=ag_in[layer][P - 1 : P, T_PER_CORE - 1, :], in_=b_ext[:]
                )
                nc.gpsimd.collective_compute(
                    "AllGather",
                    mybir.AluOpType.bypass,
                    replica_groups=[core_ids],
                    ins=[ag_in[layer][:]],
                    outs=[g_tab[layer][:]],
                )
                if debug_taps and layer == 0:
                    nc.sync.dma_start(out=gtab_dbg[:], in_=g_tab[0][:])

                reluT = bigpool.tile([P, T_PER_CORE, P], f32, tag=f"reluT{layer}")
                for (t0, tcnt, a16, b16) in L["chunks"]:
                    gbufA = gpoolA.tile([P, GCH * CA, D], bf16, tag="gbufA")
                    niA = tcnt * CA * P
                    nc.gpsimd.dma_gather(
                        gbufA[:, : tcnt * CA, :],
                        g_tab[layer][:SPLIT, :],
                        idxm[:, a16 : a16 + niA // 16],
                        niA,
                        niA,
                        D,
                        single_packet=False,
                    )
                    pe_absorb(gbufA[:, 0, :1])
                    gbufB = gpoolB.tile([P, GCH * CB, D], bf16, tag="gbufB")
                    niB = tcnt * CB * P
                    nc.gpsimd.dma_gather(
                        gbufB[:, : tcnt * CB, :],
                        g_tab[layer][SPLIT:, :],
                        idxm[:, b16 : b16 + niB // 16],
                        niB,
                        niB,
                        D,
                        single_packet=False,
                    )
                    pe_absorb(gbufB[:, 0, :1])
                    if debug_taps and layer == 0 and t0 == 0:
                        nc.sync.dma_start(out=gbuf_dbg[:], in_=gbufA[:])

                    for ti in range(tcnt):
                        t = t0 + ti
                        pm = pp_msg.tile([P, D], f32, tag="pm")
                        plan = []
                        for j in range(CA):
                            c0 = t * cols + j
                            s2 = sqd[:, t : t + 1] if j == TBA + 1 else ewm[:, c0 : c0 + 1]
                            plan.append((c0, s2, gbufA[:, ti * CA + j, :]))
                        for j in range(CB):
                            c0 = t * cols + CA + j
                            plan.append((c0, ewm[:, c0 : c0 + 1], gbufB[:, ti * CB + j, :]))
                        for jj, (c0, s2, rhs) in enumerate(plan):
                            sw = build_sw(c0, s2)
                            nc.tensor.matmul(
                                pm[:],
                                lhsT=sw[:],
                                rhs=rhs,
                                start=(jj == 0),
                                stop=(jj == len(plan) - 1),
                            )
                        relu = wpool.tile([P, D], f32, tag="relu")
                        nc.scalar.activation(
                            relu[:],
                            pm[:],
                            mybir.ActivationFunctionType.Relu,
                            scale=dinv[:, t : t + 1],
                        )
                        ptr = pp_tr.tile([P, D], f32, tag="ptr")
                        nc.tensor.transpose(ptr[:], relu[:], ident[:])
                        nc.scalar.copy(reluT[:, t, :], ptr[:])
                reluT_prev = reluT

            # ---------- classifier head ----------
            outT = bigpool.tile([P, NPC_PAD], f32, tag="outT")
            CHT = 4
            for t0 in range(0, T_PER_CORE, CHT):
                tcnt = min(CHT, T_PER_CORE - t0)
                pc = pp_cls.tile([P, CHT * P], f32, tag="pc")
                nc.tensor.matmul(
                    pc[:, : tcnt * P],
                    lhsT=wc[:],
                    rhs=reluT_prev[:, t0 : t0 + tcnt, :],
                    start=True,
                    stop=True,
                )
                nc.scalar.activation(
                    outT[:, t0 * P : (t0 + tcnt) * P],
                    pc[:, : tcnt * P],
                    mybir.ActivationFunctionType.Identity,
                    bias=bc[:],
                )
            nc.sync.dma_start(out=out_ext[:], in_=outT[:])

    nc.finalize()
    return nc


_CACHE = {}


def _get_program(TBA, TBB, debug_taps=False):
    key = (TBA, TBB, debug_taps)
    if key not in _CACHE:
        _CACHE[key] = _build_program(TBA, TBB, debug_taps)
    return _CACHE[key]


def prepare(x, edge_index, edge_weight, W1, b1, W2, b2, Wc, bc):
    """Host prep: pack edges, build/fetch program, build per-core input maps."""
    x = np.asarray(x, f32n)
    TBA, TBB, idx16, dstl_cols, ew_cols = _pack_edges(
        np.asarray(edge_index), np.asarray(edge_weight)
    )
    nc = _get_program(TBA, TBB, debug_taps=getattr(prepare, "debug_taps", False))

    wc_pad = np.zeros((D, P), f32n)
    wc_pad[:, :N_CLS] = np.asarray(Wc, f32n)
    bc_pad = np.zeros((P, 1), f32n)
    bc_pad[:N_CLS, 0] = np.asarray(bc, f32n)
    iota_tile = np.broadcast_to(np.arange(P, dtype=f32n), (P, P)).astype(BF16)
    ones_col = np.ones((P, 1), BF16)
    ident = np.eye(P, dtype=f32n)

    in_maps = []
    for c in range(N_CORES):
        xT = np.zeros((P, NPC_PAD), f32n)
        xT[:, :NPC] = x[c * NPC : (c + 1) * NPC].T
        in_maps.append(
            {
                "xT": xT,
                "W1": np.asarray(W1, f32n),
                "W2": np.asarray(W2, f32n),
                "Wc": wc_pad,
                "bc": bc_pad,
                "b1row": np.asarray(b1, f32n).reshape(1, D).astype(BF16),
                "b2row": np.asarray(b2, f32n).reshape(1, D).astype(BF16),
                "idx16": idx16[c],
                "dstl_cols": dstl_cols[c],
                "ew_cols": ew_cols[c],
                "iota_tile": iota_tile,
                "ones_col": ones_col,
                "ident": ident,
            }
        )
    return nc, in_maps


def unshard(per_core_outT):
    out = np.empty((N_NODES, N_CLS), f32n)
    for c in range(N_CORES):
        outT = np.asarray(per_core_outT[c])
        out[c * NPC : (c + 1) * NPC] = outT[:N_CLS, :NPC].T
    return out


def kernel(x, edge_index, edge_weight, W1, b1, W2, b2, Wc, bc, _run_opts=None):
    nc, in_maps = prepare(x, edge_index, edge_weight, W1, b1, W2, b2, Wc, bc)
    opts = _run_opts or {}
    res = run_bass_kernel_spmd(nc, in_maps, list(range(N_CORES)), **opts)
    if opts:
        kernel.last_results = res
    return unshard([res.results[c]["outT"] for c in range(N_CORES)])



# revision 16
# speedup vs baseline: 1.3629x; 1.3629x over previous
"""Trainium2 Bass kernel for a 2-layer GCN (PyG GCNConv semantics) + linear head.

Strategy (8 NeuronCores, SPMD; v2 — precomputed S_w, no-padding edge blocks):
  - Nodes sharded by id: core c owns rows [c*6250, (c+1)*6250), padded to
    6272 = 49*128 local rows.
  - Per layer each core computes g = scale(x_shard @ W) on its own slice,
    casts to bf16 and AllGathers the 50176-row g-table to DRAM.
  - Edges (grouped per dst core) are sorted by (gather-chunk, src-half, dst)
    and cut into dense 128-edge blocks with NO per-tile padding; a block that
    straddles a dst-tile boundary is applied twice with two masked one-hot
    matrices. The one-hot "segment matrices" S_w[e, n] = ew_e * (dst_e == n)
    are PRECOMPUTED ON HOST (pure scatter of input values = layout work),
    stored bf16 in DRAM and streamed in per chunk, so the vector engine does
    no per-block work at all.
  - g[src] rows are fetched with the Q7 dma_gather (int16 indices; the table
    is addressed as two 25088-row halves; each chunk does one gather call per
    half, trailing pad indices are -1 and get trimmed by the ucode).
  - Aggregation per dst tile t: psum = Ident@g_local[t] (self loop)
      + sqd_row[t]^T ⊗ b_row (rank-1 bias) + sum_b S_w[b]^T @ gathered[b].
  - Degree: host repacks edge weights into a [128, 49*W] per-dst-lane layout
    (layout only); deg = DVE free-dim reduce + 1. dinv = sqrt(1/deg),
    sqd = sqrt(deg), dinv2 = 1/deg.
  - Scale folding: layer-1 epilogue stores relu(psum) UNSCALED (relu(d*x) =
    d*relu(x)); the missing dinv_dst of layer 1 and the dinv of g2 combine
    into gstage2 = dinv2 * (reluT @ W2). Layer-2 epilogue applies scale=dinv.
  - Head: logitsT = Wc^T @ relu2T per 4-tile chunk, + bc, one DMA out.

Host does only integer/layout work (sort, bucket, scatter of input floats
into S_w/deg layouts, index packing); all float arithmetic runs on device.
"""

import sys

import numpy as np

for _p in ("/opt/trn_rl_repo",):
    if _p not in sys.path:
        sys.path.append(_p)

import ml_dtypes

import concourse.bacc as bacc
import concourse.mybir as mybir
import concourse.tile as tile
from concourse.bass_utils import run_bass_kernel_spmd

BF16 = ml_dtypes.bfloat16

N_NODES = 50000
N_EDGES = 600000
D = 128
N_CLS = 10
N_CORES = 8
NPC = N_NODES // N_CORES  # 6250
P = 128
T_PER_CORE = (NPC + P - 1) // P  # 49
NPC_PAD = T_PER_CORE * P  # 6272
N_TAB = N_CORES * NPC_PAD  # 50176
SPLIT = 4 * NPC_PAD  # 25088: table half boundary (fits int16 indexing)
GCH = 5  # dst tiles per gather chunk
DEG_W = 40  # columns per tile in the ew_deg layout (max node in-degree pad)

f32 = mybir.dt.float32
bf16 = mybir.dt.bfloat16
i16 = mybir.dt.int16

f32n = np.float32


def _table_row(global_node):
    """Row in the AllGather'd g-table for a global node id (see gstage DMA
    order: local row r = t*128 + p lands at flat c*6272 + p*49 + t)."""
    c = global_node // NPC
    r = global_node % NPC
    return c * NPC_PAD + (r % P) * T_PER_CORE + r // P


def _pad32(n):
    return -(-n // 32) * 32


def _pack_edges(edge_index, edge_weight):
    """Builds the per-core static schedule + gather/S_w/deg payloads.

    Returns (sched, per_core) where sched is the structural schedule (same
    for the program builder) and per_core[c] holds idx16 / sw / ew_deg arrays.
    """
    src = edge_index[0].astype(np.int64)
    dst = edge_index[1].astype(np.int64)
    ew = np.asarray(edge_weight, f32n)

    tr = _table_row(src)
    half = (tr >= SPLIT).astype(np.int64)
    row_rel = tr - half * SPLIT

    core_of = dst // NPC
    dst_loc = dst % NPC

    chunk_starts = list(range(0, T_PER_CORE, GCH))
    n_chunks = len(chunk_starts)

    # Per-core max in-degree governs DEG_W validity.
    per_core = []
    chunks_meta = None  # structural schedule; must be identical across cores
    # to emit ONE SPMD program -> use per-core max block counts and pad the
    # schedule (blocks, mats) to the max so the program is shared.

    # First pass: compute per (core, chunk, half) block counts and per
    # (core, chunk) mat counts, to size the shared schedule.
    edges_by_core = []
    for c in range(N_CORES):
        m = core_of == c
        edges_by_core.append(
            (row_rel[m], half[m], dst_loc[m], ew[m].astype(f32n))
        )

    nblk = np.zeros((N_CORES, n_chunks, 2), np.int64)
    for c in range(N_CORES):
        rr, hh, dl, _ = edges_by_core[c]
        t_of = dl // P
        ch_of = t_of // GCH
        for ci in range(n_chunks):
            for h in (0, 1):
                cnt = int(((ch_of == ci) & (hh == h)).sum())
                nblk[c, ci, h] = -(-cnt // P)
    NBLK = nblk.max(axis=0)  # [n_chunks, 2] shared block counts

    # idx16 layout offsets per (chunk, half)
    idx_off = np.zeros((n_chunks, 2), np.int64)
    pos = 0
    for ci in range(n_chunks):
        for h in (0, 1):
            idx_off[ci, h] = pos
            pos += _pad32(int(NBLK[ci, h]) * 8)
    idxw = pos

    # Shared matmul schedule. For each chunk, tile-major:
    #   per tile: [self, bias] + [(half, blk, seg) ...]
    # A (half, blk) pair overlaps tile t if any lane could target it. With
    # per-core varying edge spreads, force a shared overlap structure:
    # every block is given ceil assignments: we compute, per (chunk, half,
    # blk), the set of tiles it must serve = union over cores. Extra
    # (tile, blk) pairs for cores where the block has no such lanes simply
    # get an all-zero S_w (wasted matmul but shared program).
    overlap = {}
    for c in range(N_CORES):
        rr, hh, dl, we = edges_by_core[c]
        t_of = dl // P
        ch_of = t_of // GCH
        for ci in range(n_chunks):
            for h in (0, 1):
                m = (ch_of == ci) & (hh == h)
                if not m.any():
                    continue
                order = np.argsort(dl[m], kind="stable")
                tt = t_of[m][order]
                for b in range(int(NBLK[ci, h])):
                    seg = tt[b * P : (b + 1) * P]
                    key = (ci, h, b)
                    s = overlap.setdefault(key, set())
                    for t in np.unique(seg):
                        s.add(int(t))

    sw_off = []  # per chunk: offset into sw stream (in mats)
    sched = []  # per chunk: list of per-tile mat lists
    n_mats = 0
    for ci, t0 in enumerate(chunk_starts):
        t1 = min(t0 + GCH, T_PER_CORE)
        tiles = list(range(t0, t1))
        sw_off.append(n_mats)
        per_tile = []
        for t in tiles:
            mats = []
            for h in (0, 1):
                for b in range(int(NBLK[ci, h])):
                    if t in overlap.get((ci, h, b), ()):
                        mats.append((h, b, n_mats))
                        n_mats += 1
            per_tile.append((t, mats))
        sched.append(per_tile)

    # Second pass: fill per-core payloads.
    for c in range(N_CORES):
        rr, hh, dl, we = edges_by_core[c]
        t_of = dl // P
        ch_of = t_of // GCH

        idx16 = np.full((P, idxw), -1, np.int16)
        sw = np.zeros((n_mats, P, P), f32n)
        ew_deg = np.zeros((P, T_PER_CORE * DEG_W), f32n)

        # degree layout: per dst node lane, scatter its edge weights
        lane_all = (dl % P).astype(np.int64)
        order_n = np.lexsort((np.arange(len(dl)), dl))
        dl_s = dl[order_n]
        we_s = we[order_n]
        lane_s = lane_all[order_n]
        t_s = t_of[order_n]
        # position within each node's run
        node_starts = np.concatenate(([0], np.cumsum(np.bincount(dl_s, minlength=NPC))[:-1]))
        within = np.arange(len(dl_s)) - node_starts[dl_s]
        assert within.max() < DEG_W, f"DEG_W too small: {within.max()}"
        ew_deg[lane_s, t_s * DEG_W + within] = we_s

        for ci in range(n_chunks):
            # map (h, blk, tile) -> mat id from shared schedule
            max_nb = max(int(NBLK[ci, 0]), int(NBLK[ci, 1]), 1)
            midmap = np.full((2, max_nb, T_PER_CORE), -1, np.int64)
            for t, mats in sched[ci]:
                for (h2, b2, mid) in mats:
                    midmap[h2, b2, t] = mid
            for h in (0, 1):
                m = (ch_of == ci) & (hh == h)
                nb = int(NBLK[ci, h])
                if nb == 0:
                    continue
                idxs = np.where(m)[0]
                order = np.argsort(dl[m], kind="stable")
                idxs = idxs[order]
                n_e = len(idxs)
                # pad with row 0 (S_w columns for pad lanes are zero); the
                # SPMD program shares one num_idxs_reg across cores, so the
                # trailing-negative trim path can't be used here.
                rows_pad = np.zeros(nb * P, np.int64)
                rows_pad[:n_e] = rr[idxs]
                # idx16 packing: [16, nb*8] block replicated to 8 groups;
                # position [p%16, blk*8 + p//16] holds row of (lane p, blk)
                base = rows_pad.reshape(nb, P)  # [blk, lane]
                blkarr = np.full((16, _pad32(nb * 8)), -1, np.int16)
                lanes = np.arange(P)
                for b in range(nb):
                    blkarr[lanes % 16, b * 8 + lanes // 16] = base[b].astype(np.int16)
                off = int(idx_off[ci, h])
                idx16[:, off : off + blkarr.shape[1]] = np.tile(blkarr, (8, 1))

                # S_w fill (vectorized scatter of input ew values)
                lane_in_blk = np.arange(n_e) % P
                blk_of = np.arange(n_e) // P
                d_loc = dl[idxs]
                t_e = d_loc // P
                n_rel = d_loc % P
                mid_e = midmap[h, blk_of, t_e]
                assert (mid_e >= 0).all()
                sw[mid_e, lane_in_blk, n_rel] = we[idxs]

        # DRAM sw layout: [128, n_mats*128]; row p = concat of mats' row p
        sw_dram = np.ascontiguousarray(sw.transpose(1, 0, 2)).reshape(P, n_mats * P)
        per_core.append(
            dict(idx16=idx16, sw=sw_dram.astype(BF16), ew_deg=ew_deg)
        )

    meta = dict(
        NBLK=NBLK.tolist(),
        idx_off=idx_off.tolist(),
        idxw=idxw,
        sw_off=sw_off,
        n_mats=n_mats,
        sched=sched,
        chunk_starts=chunk_starts,
    )
    return meta, per_core


def _build_program(meta):
    NBLK = meta["NBLK"]
    idx_off = meta["idx_off"]
    idxw = meta["idxw"]
    n_mats = meta["n_mats"]
    sched = meta["sched"]
    sw_off = meta["sw_off"]
    n_chunks = len(meta["chunk_starts"])

    NBA_MAX = max(nb[0] for nb in NBLK)
    NBB_MAX = max(nb[1] for nb in NBLK)
    SW_MAX = max(
        (sw_off[ci + 1] if ci + 1 < n_chunks else n_mats) - sw_off[ci]
        for ci in range(n_chunks)
    )

    nc = bacc.Bacc(target_bir_lowering=False)

    xT_ext = nc.declare_dram_parameter("xT", [P, NPC_PAD], f32, isOutput=False)
    w1_ext = nc.declare_dram_parameter("W1", [D, D], f32, isOutput=False)
    w2_ext = nc.declare_dram_parameter("W2", [D, D], f32, isOutput=False)
    wc_ext = nc.declare_dram_parameter("Wc", [D, P], f32, isOutput=False)
    bc_ext = nc.declare_dram_parameter("bc", [P, 1], f32, isOutput=False)
    b1_ext = nc.declare_dram_parameter("b1row", [1, D], f32, isOutput=False)
    b2_ext = nc.declare_dram_parameter("b2row", [1, D], f32, isOutput=False)
    idx_ext = nc.declare_dram_parameter("idx16", [P, idxw], i16, isOutput=False)
    sw_ext = nc.declare_dram_parameter("sw", [P, n_mats * P], bf16, isOutput=False)
    ewdeg_ext = nc.declare_dram_parameter(
        "ew_deg", [P, T_PER_CORE * DEG_W], f32, isOutput=False
    )
    identf_ext = nc.declare_dram_parameter("identf", [P, P], f32, isOutput=False)
    identb_ext = nc.declare_dram_parameter("identb", [P, P], bf16, isOutput=False)
    ones_ext = nc.declare_dram_parameter("ones_row", [1, P], f32, isOutput=False)
    out_ext = nc.declare_dram_parameter("outT", [P, NPC_PAD], f32, isOutput=True)

    ag_in = [
        nc.dram_tensor(f"ag_in{l}", [P, T_PER_CORE, P], bf16, kind="Internal")
        for l in (1, 2)
    ]
    g_tab = [
        nc.dram_tensor(
            f"g_table{l}", [N_TAB, D], bf16, kind="Internal", addr_space="Shared"
        )
        for l in (1, 2)
    ]

    core_ids = list(range(N_CORES))

    with tile.TileContext(nc) as tc:
        with (
            tc.tile_pool(name="const", bufs=1) as cpool,
            tc.tile_pool(name="meta", bufs=1) as mpool,
            tc.tile_pool(name="big", bufs=1) as bigpool,
            tc.tile_pool(name="gatherA", bufs=2) as gpoolA,
            tc.tile_pool(name="gatherB", bufs=2) as gpoolB,
            tc.tile_pool(name="swst", bufs=2) as swpool,
            tc.tile_pool(name="work", bufs=4) as wpool,
            tc.tile_pool(name="psum_msg", bufs=3, space="PSUM") as pp_msg,
            tc.tile_pool(name="psum_h", bufs=2, space="PSUM") as pp_h,
            tc.tile_pool(name="psum_tr", bufs=1, space="PSUM") as pp_tr,
            tc.tile_pool(name="psum_cls", bufs=1, space="PSUM") as pp_cls,
            tc.tile_pool(name="psum_trash", bufs=1, space="PSUM") as pp_trash,
        ):
            # ---------- load constants / metadata ----------
            xT = bigpool.tile([P, NPC_PAD], f32, tag="xT")
            nc.sync.dma_start(out=xT[:], in_=xT_ext[:])
            w1 = cpool.tile([D, D], f32, tag="w1")
            nc.sync.dma_start(out=w1[:], in_=w1_ext[:])
            w2 = cpool.tile([D, D], f32, tag="w2")
            nc.sync.dma_start(out=w2[:], in_=w2_ext[:])
            wc = cpool.tile([D, P], f32, tag="wc")
            nc.sync.dma_start(out=wc[:], in_=wc_ext[:])
            bc = cpool.tile([P, 1], f32, tag="bc")
            nc.sync.dma_start(out=bc[:], in_=bc_ext[:])
            b1row = cpool.tile([1, D], f32, tag="b1row")
            nc.sync.dma_start(out=b1row[:], in_=b1_ext[:])
            b2row = cpool.tile([1, D], f32, tag="b2row")
            nc.sync.dma_start(out=b2row[:], in_=b2_ext[:])
            identf = cpool.tile([P, P], f32, tag="identf")
            nc.sync.dma_start(out=identf[:], in_=identf_ext[:])
            identb = cpool.tile([P, P], bf16, tag="identb")
            nc.sync.dma_start(out=identb[:], in_=identb_ext[:])
            ones_row = cpool.tile([1, P], f32, tag="ones_row")
            nc.sync.dma_start(out=ones_row[:], in_=ones_ext[:])
            idxm = mpool.tile([P, idxw], i16, tag="idxm")
            nc.sync.dma_start(out=idxm[:], in_=idx_ext[:])
            ewdeg = mpool.tile([P, T_PER_CORE, DEG_W], f32, tag="ewdeg")
            nc.sync.dma_start(out=ewdeg[:], in_=ewdeg_ext[:])

            # PE semaphore-wait absorption (PE is hw-decoded, 1 wait max).
            n_absorb = 8 + 3 * 2 * n_chunks
            trash = pp_trash.tile([1, 1], f32, tag="trash")
            absorb_state = {"i": 0}

            def pe_absorb(ap):
                i = absorb_state["i"]
                absorb_state["i"] += 1
                nc.tensor.matmul(
                    trash[:],
                    lhsT=ap,
                    rhs=ap,
                    start=(i == 0),
                    stop=(i == n_absorb - 1),
                    skip_group_check=True,
                )

            for _t in (xT, w1, w2, identf, identb, b1row, b2row, ones_row):
                pe_absorb(_t[:1, :1])

            # DVE waits are capped at two; pre-observe ewdeg's DMA lane.
            dabs = wpool.tile([1, 1], f32, tag="dabs")
            nc.vector.tensor_copy(out=dabs[:], in_=ewdeg[:1, :1, 0])

            # ---------- degree ----------
            deg = cpool.tile([P, T_PER_CORE], f32, tag="deg")
            deg0 = wpool.tile([P, T_PER_CORE], f32, tag="deg0")
            nc.vector.tensor_reduce(
                out=deg0[:], in_=ewdeg[:], axis=mybir.AxisListType.X,
                op=mybir.AluOpType.add,
            )
            nc.vector.tensor_scalar(
                out=deg[:], in0=deg0[:], scalar1=1.0, scalar2=None,
                op0=mybir.AluOpType.add,
            )
            dinv2 = cpool.tile([P, T_PER_CORE], f32, tag="dinv2")
            nc.vector.reciprocal(out=dinv2[:], in_=deg[:])
            dinv = cpool.tile([P, T_PER_CORE], f32, tag="dinv")
            nc.scalar.activation(dinv[:], dinv2[:], mybir.ActivationFunctionType.Sqrt)
            sqd = cpool.tile([P, T_PER_CORE], f32, tag="sqd")
            nc.scalar.activation(sqd[:], deg[:], mybir.ActivationFunctionType.Sqrt)

            # Bias broadcast tiles: B_l = ones_row^T ⊗ b_l (rank-1 matmul),
            # consumed by the DVE epilogue.
            Bb = []
            for brow in (b1row, b2row):
                pb = pp_h.tile([P, D], f32, tag="ph")
                nc.tensor.matmul(
                    pb[:], lhsT=ones_row[:], rhs=brow[:], start=True, stop=True
                )
                bt = cpool.tile([P, D], f32, tag=f"Bb{len(Bb)}")
                nc.scalar.copy(bt[:], pb[:])
                Bb.append(bt)

            # ---------- layers ----------
            # layer-2 lhsT is bf16 (reluT), so W2 needs a bf16 copy on device
            w2b = cpool.tile([D, D], bf16, tag="w2b")
            nc.scalar.copy(w2b[:], w2[:])

            reluT_prev = None
            for layer in (0, 1):
                w = (w1, w2b)[layer]
                brow = (b1row, b2row)[layer]
                scale_g = (dinv, dinv2)[layer]

                gstage = bigpool.tile([P, T_PER_CORE, P], bf16, tag=f"gstage{layer}")
                for t in range(T_PER_CORE):
                    ph = pp_h.tile([P, D], f32, tag="ph")
                    lhsT = (
                        xT[:, t * P : (t + 1) * P]
                        if layer == 0
                        else reluT_prev[:, t, :]
                    )
                    nc.tensor.matmul(ph[:], lhsT=lhsT, rhs=w[:], start=True, stop=True)
                    nc.scalar.activation(
                        gstage[:, t, :],
                        ph[:],
                        mybir.ActivationFunctionType.Copy,
                        scale=scale_g[:, t : t + 1],
                    )
                nc.sync.dma_start(out=ag_in[layer][:], in_=gstage[:])
                nc.gpsimd.collective_compute(
                    "AllGather",
                    mybir.AluOpType.bypass,
                    replica_groups=[core_ids],
                    ins=[ag_in[layer][:]],
                    outs=[g_tab[layer][:]],
                )

                reluT = bigpool.tile([P, T_PER_CORE, P], bf16, tag=f"reluT{layer}")
                for ci in range(n_chunks):
                    nbA, nbB = NBLK[ci]
                    swlo = sw_off[ci]
                    swhi = sw_off[ci + 1] if ci + 1 < n_chunks else n_mats
                    nsw = swhi - swlo
                    swt = swpool.tile([P, SW_MAX, P], bf16, tag="swt")
                    nc.sync.dma_start(
                        out=swt[:, :nsw, :],
                        in_=sw_ext[:, swlo * P : swhi * P],
                    )
                    pe_absorb(swt[:1, 0, :1])

                    gbufA = gpoolA.tile([P, NBA_MAX, D], bf16, tag="gbufA")
                    if nbA:
                        niA = nbA * P
                        nc.gpsimd.dma_gather(
                            gbufA[:, :nbA, :],
                            g_tab[layer][:SPLIT, :],
                            idxm[:, idx_off[ci][0] : idx_off[ci][0] + niA // 16],
                            niA,
                            niA,
                            D,
                            single_packet=False,
                        )
                    pe_absorb(gbufA[:, 0, :1])
                    gbufB = gpoolB.tile([P, NBB_MAX, D], bf16, tag="gbufB")
                    if nbB:
                        niB = nbB * P
                        nc.gpsimd.dma_gather(
                            gbufB[:, :nbB, :],
                            g_tab[layer][SPLIT:, :],
                            idxm[:, idx_off[ci][1] : idx_off[ci][1] + niB // 16],
                            niB,
                            niB,
                            D,
                            single_packet=False,
                        )
                    pe_absorb(gbufB[:, 0, :1])

                    for (t, mats) in sched[ci]:
                        pm = pp_msg.tile([P, D], f32, tag="pm")
                        # self loop: psum += Ident^T @ g_local[t]
                        nc.tensor.matmul(
                            pm[:], lhsT=identb[:], rhs=gstage[:, t, :],
                            start=True, stop=(len(mats) == 0),
                        )
                        for jj, (h, b, mid) in enumerate(mats):
                            rhs = (gbufA if h == 0 else gbufB)[:, b, :]
                            nc.tensor.matmul(
                                pm[:],
                                lhsT=swt[:, mid - swlo, :],
                                rhs=rhs,
                                start=False,
                                stop=(jj == len(mats) - 1),
                            )
                        pre = wpool.tile([P, D], f32, tag="pre")
                        if layer == 0:
                            # pre = sqd ⊙ B1 + psum  (stored-unscaled bias form)
                            nc.vector.scalar_tensor_tensor(
                                out=pre[:], in0=Bb[0][:],
                                scalar=sqd[:, t : t + 1], in1=pm[:],
                                op0=mybir.AluOpType.mult, op1=mybir.AluOpType.add,
                            )
                        else:
                            # pre = dinv ⊙ psum + B2
                            nc.vector.scalar_tensor_tensor(
                                out=pre[:], in0=pm[:],
                                scalar=dinv[:, t : t + 1], in1=Bb[1][:],
                                op0=mybir.AluOpType.mult, op1=mybir.AluOpType.add,
                            )
                        relu = wpool.tile([P, D], f32, tag="relu")
                        nc.scalar.activation(
                            relu[:], pre[:], mybir.ActivationFunctionType.Relu,
                        )
                        ptr = pp_tr.tile([P, D], f32, tag="ptr")
                        nc.tensor.transpose(ptr[:], relu[:], identf[:])
                        nc.scalar.copy(reluT[:, t, :], ptr[:])
                reluT_prev = reluT

            # ---------- classifier head ----------
            wcb = cpool.tile([D, P], bf16, tag="wcb")
            nc.scalar.copy(wcb[:], wc[:])
            CHT = 4
            for t0 in range(0, T_PER_CORE, CHT):
                tcnt = min(CHT, T_PER_CORE - t0)
                pc = pp_cls.tile([P, CHT * P], f32, tag="pc")
                nc.tensor.matmul(
                    pc[:, : tcnt * P],
                    lhsT=wcb[:],
                    rhs=reluT_prev[:, t0 : t0 + tcnt, :],
                    start=True,
                    stop=True,
                )
                oc = wpool.tile([P, CHT * P], f32, tag="oc")
                nc.scalar.activation(
                    oc[:, : tcnt * P],
                    pc[:, : tcnt * P],
                    mybir.ActivationFunctionType.Identity,
                    bias=bc[:],
                )
                nc.sync.dma_start(
                    out=out_ext[:, t0 * P : (t0 + tcnt) * P], in_=oc[:, : tcnt * P]
                )

    nc.finalize()
    return nc


_CACHE = {}


def _get_program(meta):
    key = (
        tuple(tuple(x) for x in meta["NBLK"]),
        meta["n_mats"],
        str(meta["sched"]),
    )
    if key not in _CACHE:
        _CACHE[key] = _build_program(meta)
    return _CACHE[key]


def prepare(x, edge_index, edge_weight, W1, b1, W2, b2, Wc, bc):
    x = np.asarray(x, f32n)
    meta, per_core = _pack_edges(np.asarray(edge_index), np.asarray(edge_weight))
    nc = _get_program(meta)

    wc_pad = np.zeros((D, P), f32n)
    wc_pad[:, :N_CLS] = np.asarray(Wc, f32n)
    bc_pad = np.zeros((P, 1), f32n)
    bc_pad[:N_CLS, 0] = np.asarray(bc, f32n)
    identf = np.eye(P, dtype=f32n)
    identb = np.eye(P, dtype=f32n).astype(BF16)

    in_maps = []
    for c in range(N_CORES):
        xT = np.zeros((P, NPC_PAD), f32n)
        xT[:, :NPC] = x[c * NPC : (c + 1) * NPC].T
        in_maps.append(
            {
                "xT": xT,
                "W1": np.asarray(W1, f32n),
                "W2": np.asarray(W2, f32n),
                "Wc": wc_pad,
                "bc": bc_pad,
                "b1row": np.asarray(b1, f32n).reshape(1, D),
                "b2row": np.asarray(b2, f32n).reshape(1, D),
                "idx16": per_core[c]["idx16"],
                "sw": per_core[c]["sw"],
                "ew_deg": per_core[c]["ew_deg"],
                "identf": identf,
                "identb": identb,
                "ones_row": np.ones((1, P), f32n),
            }
        )
    return nc, in_maps


def unshard(per_core_outT):
    out = np.empty((N_NODES, N_CLS), f32n)
    for c in range(N_CORES):
        outT = np.asarray(per_core_outT[c])
        out[c * NPC : (c + 1) * NPC] = outT[:N_CLS, :NPC].T
    return out


def kernel(x, edge_index, edge_weight, W1, b1, W2, b2, Wc, bc, _run_opts=None):
    nc, in_maps = prepare(x, edge_index, edge_weight, W1, b1, W2, b2, Wc, bc)
    opts = _run_opts or {}
    res = run_bass_kernel_spmd(nc, in_maps, list(range(N_CORES)), **opts)
    if opts:
        kernel.last_results = res
    return unshard([res.results[c]["outT"] for c in range(N_CORES)])
